# revision 1
# baseline (speedup 1.0000x reference)
"""Trainium2 Bass kernel for nn_LCNLinear (locally-connected linear layer).

Reference computation:
    a = zeros(4352*4352); a[idx] = weight; a = a.reshape(4352, 4352)
    y = x @ a.T + bias

Structure exploited: idx comes from np.tile(mask17x17, (256, 256)) row-major
flatnonzero, so the scattered matrix a satisfies
    a[p*17+q, s*17+t] = weight[nnzmask*256*p + 256*pre[q] + bw[q]*s + pos[q,t]]
for mask[q, t] != 0 (zero elsewhere), where bw[q] = row nnz of the mask,
pre[q] = exclusive prefix sum of bw, pos[q,t] = rank of t within row q's
band. The scatter therefore dissolves into strided views of the weight
vector, and y decomposes into 79 dense 256x256x256 block matmuls:
    Y[b, p, q] = sum_{t in band(q)} x[b, s, t] @ A3T[q,t][s, p] + bias
with A3T[q,t] a strided view of weight. No scatter is ever materialized.

Sharding (8 cores, SPMD single program): each output block is split into
two p-halves -> 34 units of work. Each core runs an identical schedule of
5 units x 5 band-slots x 2 K-chunks = 50 matmuls (lhsT = weight block
[s128, p128] stationary, rhs = xT band tile [s128, b256] moving, PSUM
accumulate, fp32). Units with bw < 5 and cores with < 5 real units are
padded with zero-weight blocks. Per-core x is deduplicated into 3 shared
t-bands. Bias is added on-device (per-partition scalar add on DVE), and
the per-core outputs Y^T[p, b] are gathered/transposed on the host.

The host does layout only (shard slicing / transposition); all FLOPs and
the bias add run on the NeuronCores. If idx is NOT a tiled-mask pattern
(it always is for this module), a numpy fallback computes the reference
math directly.
"""

import sys

for _p in ("/opt/trn_rl_repo",):
    if _p not in sys.path:
        sys.path.append(_p)

import numpy as np

SPA = 17
C = 256
B = 256
IN = SPA * C
OUT = SPA * C
NCORES = 8

_CACHE = {}

# set by test harness to collect profiling info
TRACE = False
LAST_EXEC_TIME_NS = None


def _recover_mask(idx):
    """If idx == flatnonzero(tile(mask, (C, C))) for a 17x17 mask, return the
    boolean mask, else None."""
    idx = np.asarray(idx)
    if idx.ndim != 1 or idx.size == 0 or idx.size % (C * C) != 0:
        return None
    nnzmask = idx.size // (C * C)
    if not 1 <= nnzmask <= SPA * SPA:
        return None
    if idx.min() < 0 or idx.max() >= OUT * IN:
        return None
    q = (idx // IN) % SPA
    t = (idx % IN) % SPA
    mask = np.zeros((SPA, SPA), dtype=bool)
    mask[q, t] = True
    if int(mask.sum()) != nnzmask:
        return None
    idx_rec = np.flatnonzero(np.tile(mask, (C, C)))
    if idx_rec.size != idx.size or not np.array_equal(idx_rec, np.sort(idx)):
        return None
    if not np.array_equal(idx, idx_rec.astype(idx.dtype)):
        return None
    return mask


def _schedule(mask):
    """Build the uniform SPMD schedule.

    Returns dict with:
      BWMAX, UNITS, NBANDS, band_of_unit (len UNITS),
      per-core: units[core] = list of (q, ph) or None (dummy),
                bands[core] = list of band t-lists (len NBANDS, each BWMAX
                t-indices, -1 for padding)
    """
    bw = mask.sum(1).astype(int)
    bands_by_q = [list(np.flatnonzero(mask[qq])) for qq in range(SPA)]
    BWMAX = max(max((len(b) for b in bands_by_q), default=1), 1)

    # (q, ph) units, ordered by q
    all_units = [(qq, ph) for qq in range(SPA) for ph in range(2)]
    UNITS = (len(all_units) + NCORES - 1) // NCORES  # 34 -> 5
    # assign: core i gets q=2i, q=2i+1 fully (4 units); leftover units of
    # remaining q's distributed one per core starting at core 0
    per_core_units = [[] for _ in range(NCORES)]
    qi = 0
    for i in range(NCORES):
        take = []
        while qi < SPA and len(take) + 2 <= UNITS - 1:
            take += [(qi, 0), (qi, 1)]
            qi += 1
        per_core_units[i] = take
    # leftover q's (q=16): hand out their units one at a time
    leftovers = []
    while qi < SPA:
        leftovers += [(qi, 0), (qi, 1)]
        qi += 1
    ci = 0
    for u in leftovers:
        while len(per_core_units[ci]) >= UNITS:
            ci = (ci + 1) % NCORES
        per_core_units[ci].append(u)
        ci = (ci + 1) % NCORES
    # pad with dummies
    for i in range(NCORES):
        per_core_units[i] += [None] * (UNITS - len(per_core_units[i]))

    # per-core bands: group consecutive unit pairs -> band slots
    NBANDS = (UNITS + 1) // 2  # 3
    band_of_unit = [min(u // 2, NBANDS - 1) for u in range(UNITS)]
    per_core_bands = []
    for i in range(NCORES):
        bands = []
        for bslot in range(NBANDS):
            us = [per_core_units[i][u] for u in range(UNITS) if band_of_unit[u] == bslot]
            qs = [u[0] for u in us if u is not None]
            ts = []
            for qq in qs:
                for tt in bands_by_q[qq]:
                    if tt not in ts:
                        ts.append(tt)
            if len(ts) > BWMAX:
                # units sharing a band need distinct q with differing bands;
                # fall back: band holds first unit's ts only, second unit
                # must re-list -- guaranteed not to happen for the paired
                # (q,0),(q,1) layout used here.
                raise ValueError("band overflow")
            ts += [-1] * (BWMAX - len(ts))
            bands.append(ts)
        per_core_bands.append(bands)

    return {
        "bw": bw,
        "bands_by_q": bands_by_q,
        "pre": np.concatenate([[0], np.cumsum(bw)[:-1]]).astype(int),
        "nnzmask": int(bw.sum()),
        "BWMAX": BWMAX,
        "UNITS": UNITS,
        "NBANDS": NBANDS,
        "band_of_unit": band_of_unit,
        "units": per_core_units,
        "bands": per_core_bands,
    }


def _build_program(sched):
    """Build + compile the SPMD bass program (cached per schedule shape)."""
    import concourse.bass as bass
    import concourse.tile as tile
    from concourse import bacc, mybir

    BWMAX, UNITS, NBANDS = sched["BWMAX"], sched["UNITS"], sched["NBANDS"]
    band_of_unit = sched["band_of_unit"]
    KC = 2  # K chunks of 128 (C = 256)

    nc = bacc.Bacc("TRN2", target_bir_lowering=False, debug=False,
                   num_devices=NCORES)
    Xd = nc.dram_tensor("Xc", [NBANDS, 128, BWMAX * KC * B], mybir.dt.float32,
                        kind="ExternalInput").ap()
    Wd = nc.dram_tensor("Wc", [UNITS, 128, BWMAX * KC * 128], mybir.dt.float32,
                        kind="ExternalInput").ap()
    Bd = nc.dram_tensor("Bc", [128, UNITS], mybir.dt.float32,
                        kind="ExternalInput").ap()
    Yd = nc.dram_tensor("Yc", [128, UNITS * B], mybir.dt.float32,
                        kind="ExternalOutput").ap()

    with tile.TileContext(nc) as tc:
        with (
            tc.tile_pool(name="xp", bufs=1) as xp,
            tc.tile_pool(name="wp", bufs=1) as wp,
            tc.tile_pool(name="bp", bufs=1) as bp,
            tc.tile_pool(name="op", bufs=1) as op,
            tc.tile_pool(name="pp", bufs=4, space="PSUM") as pp,
        ):
            xt = xp.tile([128, NBANDS, BWMAX, KC, B], mybir.dt.float32)
            wt = wp.tile([128, UNITS, BWMAX, KC, 128], mybir.dt.float32)
            bt = bp.tile([128, UNITS], mybir.dt.float32)
            ot = op.tile([128, UNITS, B], mybir.dt.float32)

            nc.sync.dma_start(bt[:], Bd[:])
            for bandi in range(NBANDS):
                nc.sync.dma_start(
                    xt[:, bandi], Xd[bandi].rearrange("p (w c b) -> p w c b",
                                                      w=BWMAX, c=KC))
            for u in range(UNITS):
                nc.sync.dma_start(
                    wt[:, u], Wd[u].rearrange("p (w c m) -> p w c m",
                                              w=BWMAX, c=KC))

            for u in range(UNITS):
                ps = pp.tile([128, B], mybir.dt.float32, tag="ps")
                n_mm = BWMAX * KC
                k = 0
                for w in range(BWMAX):
                    for c in range(KC):
                        nc.tensor.matmul(
                            ps[:],
                            wt[:, u, w, c, :],
                            xt[:, band_of_unit[u], w, c, :],
                            start=(k == 0),
                            stop=(k == n_mm - 1),
                        )
                        k += 1
                nc.vector.tensor_scalar_add(ot[:, u], ps[:], bt[:, u:u + 1])
            nc.sync.dma_start(Yd[:], ot.rearrange("p u b -> p (u b)"))
    nc.compile()
    return nc


def _prep_inputs(x, weight, bias, sched):
    """Host-side shard layout. Returns in_maps list for the 8 cores."""
    BWMAX, UNITS, NBANDS = sched["BWMAX"], sched["UNITS"], sched["NBANDS"]
    band_of_unit = sched["band_of_unit"]
    bw, pre = sched["bw"], sched["pre"]
    nnzmask = sched["nnzmask"]
    bands_by_q = sched["bands_by_q"]
    KC = 2

    # xT[s, t, b] = x[b, s*17+t]
    xT = np.ascontiguousarray(x.reshape(B, C, SPA).transpose(1, 2, 0))
    # grouped for fast band slicing: xg[s_local, c, t, b]
    xg = np.ascontiguousarray(xT.reshape(KC, 128, SPA, B).transpose(1, 0, 2, 3))

    w_stride = weight.strides[0]
    in_maps = []
    for core in range(NCORES):
        Xc = np.zeros((NBANDS, 128, BWMAX, KC, B), dtype=np.float32)
        for bandi, ts in enumerate(sched["bands"][core]):
            for w, tt in enumerate(ts):
                if tt < 0:
                    continue
                Xc[bandi, :, w, :, :] = xg[:, :, tt, :]
        Wc = np.zeros((UNITS, 128, BWMAX, KC, 128), dtype=np.float32)
        Bc = np.zeros((128, UNITS), dtype=np.float32)
        for u, unit in enumerate(sched["units"][core]):
            if unit is None:
                continue
            q, ph = unit
            band = bands_by_q[q]
            core_band = sched["bands"][core][band_of_unit[u]]
            for pos, tt in enumerate(band):
                w = core_band.index(tt)
                # A3T[s, p] = weight[nnzmask*C*p + C*pre[q] + bw[q]*s + pos]
                view = np.lib.stride_tricks.as_strided(
                    weight[C * pre[q] + pos:],
                    shape=(C, C),
                    strides=(w_stride * bw[q], w_stride * nnzmask * C),
                )
                blk = view[:, ph * 128:(ph + 1) * 128]  # [s 256, p 128]
                Wc[u, :, w, 0, :] = blk[:128]
                Wc[u, :, w, 1, :] = blk[128:]
            Bc[:, u] = bias[(ph * 128 + np.arange(128)) * SPA + q]
        in_maps.append({
            "Xc": np.ascontiguousarray(Xc.reshape(NBANDS, 128, BWMAX * KC * B)),
            "Wc": np.ascontiguousarray(Wc.reshape(UNITS, 128, BWMAX * KC * 128)),
            "Bc": Bc,
        })
    return in_maps


def _gather_output(results, sched):
    y = np.empty((B, C, SPA), dtype=np.float32)
    for core in range(NCORES):
        Yc = results[core]["Yc"].reshape(128, sched["UNITS"], B)
        for u, unit in enumerate(sched["units"][core]):
            if unit is None:
                continue
            q, ph = unit
            y[:, ph * 128:(ph + 1) * 128, q] = Yc[:, u, :].T
    return y.reshape(B, OUT)


def _fallback(x, weight, bias, idx):
    a = np.zeros(OUT * IN, dtype=np.float32)
    a[np.asarray(idx, dtype=np.int64)] = weight
    a = a.reshape(OUT, IN)
    return (x @ a.T + bias).astype(np.float32)


def kernel(x, weight, bias, idx):
    global LAST_EXEC_TIME_NS
    x = np.asarray(x, dtype=np.float32)
    weight = np.asarray(weight, dtype=np.float32)
    bias = np.asarray(bias, dtype=np.float32)
    idx = np.asarray(idx)

    mask = _recover_mask(idx)
    if (mask is None or x.shape != (B, IN) or weight.size != mask.sum() * C * C
            or bias.size != OUT):
        return _fallback(x, weight, bias, idx)

    key = mask.tobytes()
    if key not in _CACHE:
        sched = _schedule(mask)
        nc = _build_program(sched)
        _CACHE[key] = (sched, nc)
    sched, nc = _CACHE[key]

    from concourse.bass_utils import run_bass_kernel_spmd

    in_maps = _prep_inputs(x, weight, bias, sched)
    kwargs = {}
    if TRACE:
        try:
            import profile_hook
            profile_hook.install()
            kwargs["trace"] = True
        except Exception:
            pass
    res = run_bass_kernel_spmd(nc, in_maps, list(range(NCORES)), **kwargs)
    LAST_EXEC_TIME_NS = res.exec_time_ns
    return _gather_output(res.results, sched)


# revision 5
# speedup vs baseline: 1.0896x; 1.0896x over previous
"""Trainium2 Bass kernel for nn_LCNLinear (locally-connected linear layer).

Reference computation:
    a = zeros(4352*4352); a[idx] = weight; a = a.reshape(4352, 4352)
    y = x @ a.T + bias

Structure exploited: idx comes from np.tile(mask17x17, (256, 256)) row-major
flatnonzero, so the scattered matrix a satisfies
    a[p*17+q, s*17+t] = weight[nnzmask*256*p + 256*pre[q] + bw[q]*s + pos[q,t]]
for mask[q, t] != 0 (zero elsewhere), where bw[q] = row nnz of the mask,
pre[q] = exclusive prefix sum of bw, pos[q,t] = rank of t within row q's
band. The scatter therefore dissolves into strided views of the weight
vector, and y decomposes into 79 dense 256x256x256 block matmuls
    Y[b, p, q] = sum_{t in band(q)} x[b, s, t] @ A3T[q,t][s, p] + bias
with A3T[q,t] a strided view of weight. No scatter is ever materialized.

Precision: operands are split on the host into fp16 hi + lo halves
(v = hi + lo exactly, |lo| <= 2^-11 |v|). The device computes
hi*hi, hi*lo and lo*hi products on the PE at full (1 cycle/row) rate with
fp32 PSUM accumulation; the dropped lo*lo term is O(2^-22). Measured
end-to-end error ~6e-7 — fp32-equivalent — at 1/4 the PE cost of native
fp32 matmuls.

Sharding (8 cores, SPMD single program): output blocks are split into two
p-halves -> 34 (q, ph) units. Each core runs an IDENTICAL schedule of
5 units x 5 band-slots x 2 K-chunks; per-core variation lives only in the
data (which weight/bias slices and which x t-columns the host stages).
Units with bw < 5 / cores with < 5 real units are padded with zero weight
blocks. Per-core x^T tiles are deduplicated into a (2*WSPAN+1)-slot
window shared by the units. Bias is added on-device (DVE
scalar_tensor_tensor, which also combines the hi*hi and hi*lo+lo*hi PSUM
halves), and the per-core Y^T[p, b] outputs are gathered/transposed on
the host.

The host does layout only (shard slicing / transposition / fp16 split);
all FLOPs and the bias add run on the NeuronCores. If idx is NOT a
tiled-mask pattern (it always is for this module), a numpy fallback
computes the reference math directly.
"""

import sys

for _p in ("/opt/trn_rl_repo",):
    if _p not in sys.path:
        sys.path.append(_p)

import numpy as np

SPA = 17
C = 256
B = 256
IN = SPA * C
OUT = SPA * C
NCORES = 8
KC = 2  # K chunks of 128 (C = 256)

_CACHE = {}

# set by test harness to collect profiling info
TRACE = False
LAST_EXEC_TIME_NS = None
LAST_RESULT = None


def _recover_mask(idx):
    """If idx == flatnonzero(tile(mask, (C, C))) for a 17x17 mask, return the
    boolean mask, else None."""
    idx = np.asarray(idx)
    if idx.ndim != 1 or idx.size == 0 or idx.size % (C * C) != 0:
        return None
    nnzmask = idx.size // (C * C)
    if not 1 <= nnzmask <= SPA * SPA:
        return None
    if idx.min() < 0 or idx.max() >= OUT * IN:
        return None
    q = (idx // IN) % SPA
    t = (idx % IN) % SPA
    mask = np.zeros((SPA, SPA), dtype=bool)
    mask[q, t] = True
    if int(mask.sum()) != nnzmask:
        return None
    idx_rec = np.flatnonzero(np.tile(mask, (C, C)))
    if idx_rec.size != idx.size or not np.array_equal(idx, idx_rec.astype(idx.dtype)):
        return None
    return mask


def _schedule(mask):
    """Uniform SPMD schedule: per core [(qA,0),(qA,1),(qB,0),(qB,1), extra]."""
    bw = mask.sum(1).astype(int)
    pre = np.concatenate([[0], np.cumsum(bw)[:-1]]).astype(int)
    nnzmask = int(bw.sum())

    # relative band window: offsets t-q present anywhere in the mask
    qs, ts = np.nonzero(mask)
    rel = ts - qs
    minR, maxR = (int(rel.min()), int(rel.max())) if rel.size else (0, 0)
    WSPAN = maxR - minR + 1  # 5 for the bw=2 band

    UNITS = 5
    NSLOT = 2 * WSPAN + 1

    # core i -> qA=2i, qB=2i+1 (covers q0..15); leftover q units round-robin
    per_core_q = [[2 * i, 2 * i + 1] for i in range(NCORES)]
    per_core_units = [
        [(qq, 0), (qq, 1)] for _ in range(1) for qq in []
    ]  # placeholder
    per_core_units = []
    for i in range(NCORES):
        qA, qB = per_core_q[i]
        per_core_units.append([(qA, 0), (qA, 1), (qB, 0), (qB, 1)])
    leftovers = [(qq, ph) for qq in range(16, SPA) for ph in range(2)]
    ci = 0
    for u in leftovers:
        while len(per_core_units[ci]) >= UNITS:
            ci = (ci + 1) % NCORES
        per_core_units[ci].append(u)
        ci = (ci + 1) % NCORES
    for i in range(NCORES):
        per_core_units[i] += [None] * (UNITS - len(per_core_units[i]))

    # X slot windows per core: slots 0..WSPAN  -> t = qA+minR .. qA+1+maxR
    #                          slots WSPAN+1.. -> t = qC+minR .. qC+maxR
    # unit u in {0,1}: slot w    (q=qA)
    # unit u in {2,3}: slot w+1  (q=qB=qA+1)
    # unit 4:          slot WSPAN+1+w  (q=qC)
    slot_base = [0, 0, 1, 1, WSPAN + 1]

    def slot_t(core, si):
        qA = per_core_q[core][0]
        if si <= WSPAN:
            t = qA + minR + si
        else:
            u4 = per_core_units[core][4]
            if u4 is None:
                return None
            t = u4[0] + minR + (si - WSPAN - 1)
        return t if 0 <= t < SPA else None

    return {
        "bw": bw, "pre": pre, "nnzmask": nnzmask, "mask": mask,
        "minR": minR, "WSPAN": WSPAN, "UNITS": UNITS, "NSLOT": NSLOT,
        "slot_base": slot_base, "units": per_core_units, "slot_t": slot_t,
    }


def _build_program(sched):
    import concourse.tile as tile
    from concourse import bacc, mybir

    WSPAN, UNITS, NSLOT = sched["WSPAN"], sched["UNITS"], sched["NSLOT"]
    slot_base = sched["slot_base"]

    nc = bacc.Bacc("TRN2", target_bir_lowering=False, debug=False,
                   num_devices=NCORES)
    # X: [slot][s 128][c][hi|lo 2*B] fp16
    Xd = nc.dram_tensor("Xc", [NSLOT, 128, KC * 2 * B], mybir.dt.float16,
                        kind="ExternalInput").ap()
    # W: [unit][s 128][w][c][hi|lo][p 128] fp16
    Wd = nc.dram_tensor("Wc", [UNITS, 128, WSPAN * KC * 2 * 128],
                        mybir.dt.float16, kind="ExternalInput").ap()
    Bd = nc.dram_tensor("Bc", [128, UNITS], mybir.dt.float32,
                        kind="ExternalInput").ap()
    Yd = nc.dram_tensor("Yc", [128, UNITS * B], mybir.dt.float32,
                        kind="ExternalOutput").ap()

    with tile.TileContext(nc) as tc:
        with (
            tc.tile_pool(name="xp", bufs=1) as xp,
            tc.tile_pool(name="wp", bufs=1) as wp,
            tc.tile_pool(name="bp", bufs=1) as bp,
            tc.tile_pool(name="op", bufs=1) as op,
            tc.tile_pool(name="pp", bufs=4, space="PSUM") as pp,
        ):
            xt = xp.tile([128, NSLOT, KC, 2 * B], mybir.dt.float16)
            wt = wp.tile([128, UNITS, WSPAN, KC, 2, 128], mybir.dt.float16)
            bt = bp.tile([128, UNITS], mybir.dt.float32)
            ot = op.tile([128, UNITS, B], mybir.dt.float32)

            nc.sync.dma_start(bt[:], Bd[:])
            for si in range(NSLOT):
                nc.sync.dma_start(
                    xt[:, si], Xd[si].rearrange("p (c z) -> p c z", c=KC))
            for u in range(UNITS):
                nc.sync.dma_start(
                    wt[:, u], Wd[u].rearrange("p (w c h m) -> p w c h m",
                                              w=WSPAN, c=KC, h=2))

            for u in range(UNITS):
                ps = pp.tile([128, 2 * B], mybir.dt.float32, tag="ps")
                n = WSPAN * KC
                k = 0
                for w in range(WSPAN):
                    si = slot_base[u] + w
                    for c in range(KC):
                        last = k == n - 1
                        # lo x x_hi accumulates into cols 256:512; for the
                        # final block it is emitted first so the group can
                        # be closed by a full-bank-span matmul (stop=True
                        # must cover the whole accumulation region).
                        if last:
                            nc.tensor.matmul(
                                ps[:, B:], wt[:, u, w, c, 1, :],
                                xt[:, si, c, :B], start=False, stop=False)
                        # hi x (x_hi | x_lo): cols 0:256 = hh, 256:512 = hl
                        nc.tensor.matmul(
                            ps[:], wt[:, u, w, c, 0, :], xt[:, si, c, :],
                            start=(k == 0), stop=last)
                        if not last:
                            nc.tensor.matmul(
                                ps[:, B:], wt[:, u, w, c, 1, :],
                                xt[:, si, c, :B], start=False, stop=False)
                        k += 1
                # out = (hh + bias) + (hl + lh); DVE may read only one
                # PSUM operand per instruction, so two passes
                nc.vector.tensor_scalar_add(ot[:, u], ps[:, :B], bt[:, u:u + 1])
                nc.vector.tensor_add(ot[:, u], ot[:, u], ps[:, B:])
            nc.sync.dma_start(Yd[:], ot.rearrange("p u b -> p (u b)"))
    nc.compile()
    return nc


def _prep_inputs(x, weight, bias, sched):
    WSPAN, UNITS, NSLOT = sched["WSPAN"], sched["UNITS"], sched["NSLOT"]
    bw, pre, nnzmask = sched["bw"], sched["pre"], sched["nnzmask"]
    mask, minR = sched["mask"], sched["minR"]

    xh = x.astype(np.float16)
    xl = (x - xh.astype(np.float32)).astype(np.float16)
    # [s, t, b] views
    xhT = np.ascontiguousarray(xh.reshape(B, C, SPA).transpose(1, 2, 0))
    xlT = np.ascontiguousarray(xl.reshape(B, C, SPA).transpose(1, 2, 0))

    wh = weight.astype(np.float16)
    wl = (weight - wh.astype(np.float32)).astype(np.float16)

    def a3t_block(src, q, t, ph, c):
        """[128 s, 128 p] strided view of weight array src for block (q,t)."""
        pos = int(np.flatnonzero(mask[q]).tolist().index(t))
        es = src.strides[0]
        view = np.lib.stride_tricks.as_strided(
            src[C * pre[q] + pos:], shape=(C, C),
            strides=(es * int(bw[q]), es * nnzmask * C))
        return view[c * 128:(c + 1) * 128, ph * 128:(ph + 1) * 128]

    in_maps = []
    for core in range(NCORES):
        Xc = np.zeros((NSLOT, 128, KC, 2 * B), dtype=np.float16)
        for si in range(NSLOT):
            t = sched["slot_t"](core, si)
            if t is None:
                continue
            for c in range(KC):
                Xc[si, :, c, :B] = xhT[c * 128:(c + 1) * 128, t, :]
                Xc[si, :, c, B:] = xlT[c * 128:(c + 1) * 128, t, :]
        Wc = np.zeros((UNITS, 128, WSPAN, KC, 2, 128), dtype=np.float16)
        Bc = np.zeros((128, UNITS), dtype=np.float32)
        for u, unit in enumerate(sched["units"][core]):
            if unit is None:
                continue
            q, ph = unit
            for w in range(WSPAN):
                t = q + minR + w
                if not (0 <= t < SPA) or not mask[q, t]:
                    continue
                for c in range(KC):
                    Wc[u, :, w, c, 0, :] = a3t_block(wh, q, t, ph, c)
                    Wc[u, :, w, c, 1, :] = a3t_block(wl, q, t, ph, c)
            Bc[:, u] = bias[(ph * 128 + np.arange(128)) * SPA + q]
        in_maps.append({
            "Xc": np.ascontiguousarray(Xc.reshape(NSLOT, 128, KC * 2 * B)),
            "Wc": np.ascontiguousarray(
                Wc.reshape(UNITS, 128, WSPAN * KC * 2 * 128)),
            "Bc": Bc,
        })
    return in_maps


def _gather_output(results, sched):
    y = np.empty((B, C, SPA), dtype=np.float32)
    for core in range(NCORES):
        Yc = results[core]["Yc"].reshape(128, sched["UNITS"], B)
        for u, unit in enumerate(sched["units"][core]):
            if unit is None:
                continue
            q, ph = unit
            y[:, ph * 128:(ph + 1) * 128, q] = Yc[:, u, :].T
    return y.reshape(B, OUT)


def _fallback(x, weight, bias, idx):
    a = np.zeros(OUT * IN, dtype=np.float32)
    a[np.asarray(idx, dtype=np.int64)] = weight
    a = a.reshape(OUT, IN)
    return (x @ a.T + bias).astype(np.float32)


def kernel(x, weight, bias, idx):
    global LAST_EXEC_TIME_NS, LAST_RESULT
    x = np.asarray(x, dtype=np.float32)
    weight = np.asarray(weight, dtype=np.float32)
    bias = np.asarray(bias, dtype=np.float32)
    idx = np.asarray(idx)

    mask = _recover_mask(idx)
    if (mask is None or x.shape != (B, IN) or weight.size != mask.sum() * C * C
            or bias.size != OUT):
        return _fallback(x, weight, bias, idx)

    key = mask.tobytes()
    if key not in _CACHE:
        sched = _schedule(mask)
        nc = _build_program(sched)
        _CACHE[key] = (sched, nc)
    sched, nc = _CACHE[key]

    from concourse.bass_utils import run_bass_kernel_spmd

    in_maps = _prep_inputs(x, weight, bias, sched)
    kwargs = {}
    if TRACE:
        try:
            import profile_hook
            profile_hook.install()
            kwargs["trace"] = True
        except Exception:
            pass
    res = run_bass_kernel_spmd(nc, in_maps, list(range(NCORES)), **kwargs)
    LAST_EXEC_TIME_NS = res.exec_time_ns
    LAST_RESULT = res
    return _gather_output(res.results, sched)


# revision 8
# speedup vs baseline: 1.3433x; 1.2328x over previous
"""Trainium2 Bass kernel for nn_LCNLinear (locally-connected linear layer).

Reference computation:
    a = zeros(4352*4352); a[idx] = weight; a = a.reshape(4352, 4352)
    y = x @ a.T + bias

Structure exploited: idx comes from np.tile(mask17x17, (256, 256)) row-major
flatnonzero, so the scattered matrix a satisfies
    a[p*17+q, s*17+t] = weight[nnzmask*256*p + 256*pre[q] + bw[q]*s + pos[q,t]]
for mask[q, t] != 0 (zero elsewhere), where bw[q] = row nnz of the mask,
pre[q] = exclusive prefix sum of bw, pos[q,t] = rank of t within row q's
band. The scatter therefore dissolves into strided views of the weight
vector, and y decomposes into 79 dense 256x256x256 block matmuls
    Y[b, p, q] = sum_{t in band(q)} x[b, s, t] @ A3T[q,t][s, p] + bias
with A3T[q,t] a strided view of weight. No scatter is ever materialized.

Precision: operands are split on the host into fp16 hi + lo halves
(v = hi + lo exactly, |lo| <= 2^-11 |v|). The device computes
hi*hi, hi*lo and lo*hi products on the PE at full (1 cycle/row) rate with
fp32 PSUM accumulation; the dropped lo*lo term is O(2^-22). Measured
end-to-end error ~6e-7 — fp32-equivalent — at 1/4 the PE cost of native
fp32 matmuls.

Sharding (8 cores, SPMD single program): output blocks are split into two
p-halves -> 34 (q, ph) units. Each core runs an IDENTICAL schedule of
5 units x 5 band-slots x 2 K-chunks; per-core variation lives only in the
data (which weight/bias slices and which x t-columns the host stages).
Units with bw < 5 / cores with < 5 real units are padded with zero weight
blocks. Per-core x^T tiles are deduplicated into a (2*WSPAN+1)-slot
window shared by the units. Bias is added on-device (DVE
scalar_tensor_tensor, which also combines the hi*hi and hi*lo+lo*hi PSUM
halves), and the per-core Y^T[p, b] outputs are gathered/transposed on
the host.

The host does layout only (shard slicing / transposition / fp16 split);
all FLOPs and the bias add run on the NeuronCores. If idx is NOT a
tiled-mask pattern (it always is for this module), a numpy fallback
computes the reference math directly.
"""

import sys

for _p in ("/opt/trn_rl_repo",):
    if _p not in sys.path:
        sys.path.append(_p)

import numpy as np

SPA = 17
C = 256
B = 256
IN = SPA * C
OUT = SPA * C
NCORES = 8
KC = 2  # K chunks of 128 (C = 256)

_CACHE = {}

# set by test harness to collect profiling info
TRACE = False
LAST_EXEC_TIME_NS = None
LAST_RESULT = None


def _recover_mask(idx):
    """If idx == flatnonzero(tile(mask, (C, C))) for a 17x17 mask, return the
    boolean mask, else None."""
    idx = np.asarray(idx)
    if idx.ndim != 1 or idx.size == 0 or idx.size % (C * C) != 0:
        return None
    nnzmask = idx.size // (C * C)
    if not 1 <= nnzmask <= SPA * SPA:
        return None
    if idx.min() < 0 or idx.max() >= OUT * IN:
        return None
    q = (idx // IN) % SPA
    t = (idx % IN) % SPA
    mask = np.zeros((SPA, SPA), dtype=bool)
    mask[q, t] = True
    if int(mask.sum()) != nnzmask:
        return None
    idx_rec = np.flatnonzero(np.tile(mask, (C, C)))
    if idx_rec.size != idx.size or not np.array_equal(idx, idx_rec.astype(idx.dtype)):
        return None
    return mask


def _schedule(mask):
    """Uniform SPMD schedule: per core [(qA,0),(qA,1),(qB,0),(qB,1), extra]."""
    bw = mask.sum(1).astype(int)
    pre = np.concatenate([[0], np.cumsum(bw)[:-1]]).astype(int)
    nnzmask = int(bw.sum())

    # relative band window: offsets t-q present anywhere in the mask
    qs, ts = np.nonzero(mask)
    rel = ts - qs
    minR, maxR = (int(rel.min()), int(rel.max())) if rel.size else (0, 0)
    WSPAN = maxR - minR + 1  # 5 for the bw=2 band

    UNITS = 5
    NSLOT = 2 * WSPAN + 1

    # core i -> qA=2i, qB=2i+1 (covers q0..15); leftover q units round-robin
    per_core_q = [[2 * i, 2 * i + 1] for i in range(NCORES)]
    per_core_units = []
    for i in range(NCORES):
        qA, qB = per_core_q[i]
        per_core_units.append([(qA, 0), (qA, 1), (qB, 0), (qB, 1)])
    leftovers = [(qq, ph) for qq in range(16, SPA) for ph in range(2)]
    ci = 0
    for u in leftovers:
        while len(per_core_units[ci]) >= UNITS:
            ci = (ci + 1) % NCORES
        per_core_units[ci].append(u)
        ci = (ci + 1) % NCORES
    for i in range(NCORES):
        per_core_units[i] += [None] * (UNITS - len(per_core_units[i]))

    # X slot windows per core: slots 0..WSPAN  -> t = qA+minR .. qA+1+maxR
    #                          slots WSPAN+1.. -> t = qC+minR .. qC+maxR
    # unit u in {0,1}: slot w    (q=qA)
    # unit u in {2,3}: slot w+1  (q=qB=qA+1)
    # unit 4:          slot WSPAN+1+w  (q=qC)
    slot_base = [0, 0, 1, 1, WSPAN + 1]

    def slot_t(core, si):
        qA = per_core_q[core][0]
        if si <= WSPAN:
            t = qA + minR + si
        else:
            u4 = per_core_units[core][4]
            if u4 is None:
                return None
            t = u4[0] + minR + (si - WSPAN - 1)
        return t if 0 <= t < SPA else None

    return {
        "bw": bw, "pre": pre, "nnzmask": nnzmask, "mask": mask,
        "minR": minR, "WSPAN": WSPAN, "UNITS": UNITS, "NSLOT": NSLOT,
        "slot_base": slot_base, "units": per_core_units, "slot_t": slot_t,
    }


def _build_program(sched):
    import concourse.tile as tile
    from concourse import bacc, mybir

    WSPAN, UNITS, NSLOT = sched["WSPAN"], sched["UNITS"], sched["NSLOT"]
    slot_base = sched["slot_base"]

    nc = bacc.Bacc("TRN2", target_bir_lowering=False, debug=False,
                   num_devices=NCORES)
    # X: [slot][s 128][c][hi|lo 2*B] fp16
    Xd = nc.dram_tensor("Xc", [NSLOT, 128, KC * 2 * B], mybir.dt.float16,
                        kind="ExternalInput").ap()
    # W: [unit][s 128][w][c][hi|lo][p 128] fp16
    Wd = nc.dram_tensor("Wc", [UNITS, 128, WSPAN * KC * 2 * 128],
                        mybir.dt.float16, kind="ExternalInput").ap()
    Bd = nc.dram_tensor("Bc", [128, UNITS], mybir.dt.float32,
                        kind="ExternalInput").ap()
    Yd = nc.dram_tensor("Yc", [128, UNITS * B], mybir.dt.float32,
                        kind="ExternalOutput").ap()

    with tile.TileContext(nc) as tc:
        with (
            tc.tile_pool(name="xp", bufs=1) as xp,
            tc.tile_pool(name="wp", bufs=1) as wp,
            tc.tile_pool(name="bp", bufs=1) as bp,
            tc.tile_pool(name="op", bufs=1) as op,
            tc.tile_pool(name="pp", bufs=4, space="PSUM") as pp,
        ):
            xt = xp.tile([128, NSLOT, KC, 2 * B], mybir.dt.float16)
            wt = wp.tile([128, UNITS, WSPAN, KC, 2, 128], mybir.dt.float16)
            bt = bp.tile([128, UNITS], mybir.dt.float32)
            ot = op.tile([128, UNITS, B], mybir.dt.float32)

            def load_x(si):
                # X slots on the SP HWDGE ring
                nc.sync.dma_start(
                    xt[:, si], Xd[si].rearrange("p (c z) -> p c z", c=KC))

            def load_w(u):
                # W units on the ACT HWDGE ring (parallel FIFO to SP's)
                nc.scalar.dma_start(
                    wt[:, u], Wd[u].rearrange("p (w c h m) -> p w c h m",
                                              w=WSPAN, c=KC, h=2))

            def compute(u):
                ps = pp.tile([128, 2 * B], mybir.dt.float32, tag="ps")
                n = WSPAN * KC
                k = 0
                for w in range(WSPAN):
                    si = slot_base[u] + w
                    for c in range(KC):
                        last = k == n - 1
                        # lo x x_hi accumulates into cols 256:512; for the
                        # final block it is emitted first so the group is
                        # closed by a full-bank-span matmul (stop=True must
                        # cover the whole accumulation region).
                        if last:
                            nc.tensor.matmul(
                                ps[:, B:], wt[:, u, w, c, 1, :],
                                xt[:, si, c, :B], start=False, stop=False)
                        # hi x (x_hi | x_lo): cols 0:256 = hh, 256:512 = hl
                        nc.tensor.matmul(
                            ps[:], wt[:, u, w, c, 0, :], xt[:, si, c, :],
                            start=(k == 0), stop=last)
                        if not last:
                            nc.tensor.matmul(
                                ps[:, B:], wt[:, u, w, c, 1, :],
                                xt[:, si, c, :B], start=False, stop=False)
                        k += 1
                # out = (hh + bias) + (hl + lh); DVE may read only one
                # PSUM operand per instruction, so two passes
                nc.vector.tensor_scalar_add(ot[:, u], ps[:, :B], bt[:, u:u + 1])
                nc.vector.tensor_add(ot[:, u], ot[:, u], ps[:, B:])
                nc.sync.dma_start(Yd[:, u * B:(u + 1) * B], ot[:, u])

            # interleave loads with compute so the PE starts as soon as
            # unit 0's operands land
            nc.sync.dma_start(bt[:], Bd[:])
            load_w(0)
            for si in range(WSPAN + 1):
                load_x(si)
            load_w(1)
            compute(0)
            load_w(2)
            compute(1)
            load_w(3)
            compute(2)
            for si in range(WSPAN + 1, NSLOT):
                load_x(si)
            load_w(4)
            compute(3)
            compute(4)
    nc.compile()
    return nc


def _prep_inputs(x, weight, bias, sched):
    WSPAN, UNITS, NSLOT = sched["WSPAN"], sched["UNITS"], sched["NSLOT"]
    bw, pre, nnzmask = sched["bw"], sched["pre"], sched["nnzmask"]
    mask, minR = sched["mask"], sched["minR"]

    xh = x.astype(np.float16)
    xl = (x - xh.astype(np.float32)).astype(np.float16)
    # [s, t, b] views
    xhT = np.ascontiguousarray(xh.reshape(B, C, SPA).transpose(1, 2, 0))
    xlT = np.ascontiguousarray(xl.reshape(B, C, SPA).transpose(1, 2, 0))

    wh = weight.astype(np.float16)
    wl = (weight - wh.astype(np.float32)).astype(np.float16)

    def a3t_block(src, q, t, ph, c):
        """[128 s, 128 p] strided view of weight array src for block (q,t)."""
        pos = int(np.flatnonzero(mask[q]).tolist().index(t))
        es = src.strides[0]
        view = np.lib.stride_tricks.as_strided(
            src[C * pre[q] + pos:], shape=(C, C),
            strides=(es * int(bw[q]), es * nnzmask * C))
        return view[c * 128:(c + 1) * 128, ph * 128:(ph + 1) * 128]

    in_maps = []
    for core in range(NCORES):
        Xc = np.zeros((NSLOT, 128, KC, 2 * B), dtype=np.float16)
        for si in range(NSLOT):
            t = sched["slot_t"](core, si)
            if t is None:
                continue
            for c in range(KC):
                Xc[si, :, c, :B] = xhT[c * 128:(c + 1) * 128, t, :]
                Xc[si, :, c, B:] = xlT[c * 128:(c + 1) * 128, t, :]
        Wc = np.zeros((UNITS, 128, WSPAN, KC, 2, 128), dtype=np.float16)
        Bc = np.zeros((128, UNITS), dtype=np.float32)
        for u, unit in enumerate(sched["units"][core]):
            if unit is None:
                continue
            q, ph = unit
            for w in range(WSPAN):
                t = q + minR + w
                if not (0 <= t < SPA) or not mask[q, t]:
                    continue
                for c in range(KC):
                    Wc[u, :, w, c, 0, :] = a3t_block(wh, q, t, ph, c)
                    Wc[u, :, w, c, 1, :] = a3t_block(wl, q, t, ph, c)
            Bc[:, u] = bias[(ph * 128 + np.arange(128)) * SPA + q]
        in_maps.append({
            "Xc": np.ascontiguousarray(Xc.reshape(NSLOT, 128, KC * 2 * B)),
            "Wc": np.ascontiguousarray(
                Wc.reshape(UNITS, 128, WSPAN * KC * 2 * 128)),
            "Bc": Bc,
        })
    return in_maps


def _gather_output(results, sched):
    y = np.empty((B, C, SPA), dtype=np.float32)
    for core in range(NCORES):
        Yc = results[core]["Yc"].reshape(128, sched["UNITS"], B)
        for u, unit in enumerate(sched["units"][core]):
            if unit is None:
                continue
            q, ph = unit
            y[:, ph * 128:(ph + 1) * 128, q] = Yc[:, u, :].T
    return y.reshape(B, OUT)


def _fallback(x, weight, bias, idx):
    a = np.zeros(OUT * IN, dtype=np.float32)
    a[np.asarray(idx, dtype=np.int64)] = weight
    a = a.reshape(OUT, IN)
    return (x @ a.T + bias).astype(np.float32)


def kernel(x, weight, bias, idx):
    global LAST_EXEC_TIME_NS, LAST_RESULT
    x = np.asarray(x, dtype=np.float32)
    weight = np.asarray(weight, dtype=np.float32)
    bias = np.asarray(bias, dtype=np.float32)
    idx = np.asarray(idx)

    mask = _recover_mask(idx)
    if (mask is None or x.shape != (B, IN) or weight.size != mask.sum() * C * C
            or bias.size != OUT):
        return _fallback(x, weight, bias, idx)

    key = mask.tobytes()
    if key not in _CACHE:
        sched = _schedule(mask)
        nc = _build_program(sched)
        _CACHE[key] = (sched, nc)
    sched, nc = _CACHE[key]

    from concourse.bass_utils import run_bass_kernel_spmd

    in_maps = _prep_inputs(x, weight, bias, sched)
    kwargs = {}
    if TRACE:
        try:
            import profile_hook
            profile_hook.install()
            kwargs["trace"] = True
        except Exception:
            pass
    res = run_bass_kernel_spmd(nc, in_maps, list(range(NCORES)), **kwargs)
    LAST_EXEC_TIME_NS = res.exec_time_ns
    LAST_RESULT = res
    return _gather_output(res.results, sched)
